# revision 65
# baseline (speedup 1.0000x reference)
"""MoCo loss (InfoNCE over a 65536-entry queue + proto-NCE over 50000
k-means centroids) on 8 Trainium2 NeuronCores.

v3: fp8e4m3 operands with DoubleRowSwInterleave matmuls: the stationary
operand is pre-interleaved on the host (A/B k-layer pairs interleaved
per column, columns reversed) so the weight load reads contiguously and
stays off the critical path; the moving operand streams column pairs at
~0.5 cycles/element.  Tables are scaled by 16 per operand (dots come
out x256) and quantized to e4m3 on the host.

Per core (tables sharded by row, Z_q replicated):

  part 2 (centroid shard, zq stationary / centroids moving):
    - s2 = Z_q @ c_shard.T  (PE, fp32 acc), cast fp16 (DVE), export
      (argmax + exclusion gather on the host)
  part 1 (queue shard, queue stationary / zq moving):
    - s1 = q_shard @ Z_q.T                 (PE)
    - exp(s1/T) in fp8  (no shift; values in [e^-4, e^4])   (ACT)
    - row max over batch + s1[:,0] exported raw         (DVE)
    - ones matmul partition sum -> per-batch partial
      sum-of-exp                                            (PE)

The host combines per-core partials (logsumexp merge, global argmax,
exclusion gather + 513-wide softmax).  Accuracy is made exact despite
fp8 noise: the device exports per-queue-row (rowmax, s0) raw scores and
the host re-checks every column whose margin is below 0.05 against a
full-precision dot product (fp8 dot error is < 0.01).
"""

import os
import numpy as np
import ml_dtypes

B, C = 256, 512
QUEUE, NCL, NNEG = 65536, 50000, 512
INFO_TEMP = 0.07
PROTO_FACTOR = 0.5
NCORES = 8
QSH = QUEUE // NCORES          # 8192 queue rows per core
CSH = NCL // NCORES            # 6250 centroid rows per core
CSH_PAD = 6272                 # 14 * 448
CCH = 14                       # s2 matmul chunks
CW = CSH_PAD // CCH            # 448
KSUB = C // 128                # 4 contraction subtiles
KPAIR = KSUB // 2              # 2 DoubleRow pairs
NBT = 16                       # part-1 big tiles (512 queue rows each)
QSUBS = (8, 16, 16, 16, 8)     # qT DMA chunk sizes in 128-row subtiles
                               # (small first chunk so part 1 starts early;
                               # DMA is packet-limited: 1 pkt per line)
QOFF = (0, 8, 24, 40, 56)      # prefix offsets of QSUBS
FP8_SCALE = 16.0               # per-operand scale; dots come out x256
DOT_SCALE = FP8_SCALE * FP8_SCALE
MARGIN = 0.05                  # host re-check threshold (unscaled units)
MARGIN2 = 0.045                # part-2 argmax re-check threshold

_CACHE = {}

# exec time of the last device run (ns), populated when tracing is on
last_exec_time_ns = None


def _build():
    import concourse.bass as bass
    import concourse.tile as tile
    from concourse import bacc, mybir

    dt = mybir.dt
    DRI = mybir.MatmulPerfMode.DoubleRowSwInterleave
    nc = bacc.Bacc(
        "TRN2", target_bir_lowering=False, debug=False, num_devices=NCORES
    )

    # ---- DRAM I/O (all partition-major so every DMA is [128, N] flat) ----
    # zq packs both Z_q layouts in one transfer (DMA is packet-limited, so
    # small tensors are merged): [:, 0:4, :] = zqT (moving operand of part
    # 1, k-pair slices), [:, 4:8, :] = zqTi (part-2 stationary, host-
    # interleaved per (kpair, bt))
    zq_d = nc.dram_tensor("zq", [128, 8, 256], dt.float8e4, kind="ExternalInput").ap()
    # qT: part-1 stationary, host-interleaved per (jsub, kpair)
    qT_d = [
        nc.dram_tensor(
            f"qT{h}", [128, n, KPAIR, 256], dt.float8e4, kind="ExternalInput"
        ).ap()
        for h, n in enumerate(QSUBS)
    ]
    # cT split 3x4 + 1x2 matmul chunks for big per-partition lines
    cTa_d = nc.dram_tensor(
        "cTa", [3, 128, KSUB, 4 * CW], dt.float8e4, kind="ExternalInput"
    ).ap()
    cTb_d = nc.dram_tensor(
        "cTb", [128, KSUB, 2 * CW], dt.float8e4, kind="ExternalInput"
    ).ap()

    # single packed final export: [:, 0:64] rowmax of exp, [:, 64:128]
    # exp at batch 0, [:, 128:384] per-batch partial sum-of-exp
    fin_d = nc.dram_tensor("fin", [128, 384], dt.float32, kind="ExternalOutput").ap()
    s2_d = nc.dram_tensor(
        "s2", [2, 128, CCH, CW], dt.float8e4, kind="ExternalOutput"
    ).ap()

    with tile.TileContext(nc) as tc:
        with (
            tc.tile_pool(name="const", bufs=1) as cpool,
            tc.tile_pool(name="work", bufs=6) as wpool,
            tc.tile_pool(name="ps1", bufs=2, space="PSUM") as ps1,
            tc.tile_pool(name="psum1s", bufs=1, space="PSUM") as ps1s,
            tc.tile_pool(name="ps2", bufs=3, space="PSUM") as ps2,
        ):
            # ---- resident SBUF tensors ----
            # Issue order = HBM priority: zqT (tiny), then cT (part 2 runs
            # first), then qT.  Alternate Sync/GpSimd queues; one tile per
            # chunk (writes to a shared tile would serialize on WAW).
            # Two-queue schedule (each queue streams ~half the aggregate DMA
            # bandwidth), tuned so every chunk lands just before the PE
            # needs it; the small qt0 lands early so part-1 work is
            # available whenever part-2 stalls on a cT chunk.
            #   sync:   zq, cTb, cTa1, qt1, qt2, qt4
            #   gpsimd: cTa0, qt0, cTa2, qt3
            zq_sb = cpool.tile([128, 8, 256], dt.float8e4)
            nc.sync.dma_start(zq_sb[:], zq_d[:])

            cT_sb = [
                cpool.tile(
                    [128, KSUB, 4 * CW], dt.float8e4, name=f"cTa{ch}", tag=f"cT{ch}"
                )
                for ch in range(3)
            ]
            cTb_sb = cpool.tile([128, KSUB, 2 * CW], dt.float8e4)
            qt_sb = [
                cpool.tile(
                    [128, n, KPAIR, 256], dt.float8e4, name=f"qt{h}", tag=f"qt{h}"
                )
                for h, n in enumerate(QSUBS)
            ]

            nc.sync.dma_start(cTb_sb[:], cTb_d[:])
            nc.gpsimd.dma_start(cT_sb[0][:], cTa_d[0])
            nc.sync.dma_start(cT_sb[1][:], cTa_d[1])
            nc.gpsimd.dma_start(qt_sb[0][:], qT_d[0])
            nc.sync.dma_start(qt_sb[1][:], qT_d[1])
            nc.gpsimd.dma_start(cT_sb[2][:], cTa_d[2])
            nc.sync.dma_start(qt_sb[2][:], qT_d[2])
            nc.gpsimd.dma_start(qt_sb[3][:], qT_d[3])
            nc.sync.dma_start(qt_sb[4][:], qT_d[4])

            ones_sb = cpool.tile([128, 128], dt.bfloat16)
            nc.vector.memset(ones_sb[:], 1.0)

            # ---- part 2: centroid shard (argmax happens on the host) ----
            # fp8 export: the host re-checks near-max columns exactly, and
            # pl_neg noise averages out in the 513-wide softmax
            s2_sb = cpool.tile([128, 2, CCH, CW], dt.float8e4)

            # interleave bt inside the chunk loop: each cT chunk is consumed
            # once, right as its DMA lands (arrival rate ~ compute rate)
            for ch in range(CCH):
                if ch < 2:
                    cmov, w = cTb_sb, ch
                else:
                    dch, w = divmod(ch - 2, 4)
                    cmov = cT_sb[dch]
                for bt in range(2):
                    s2_ps = ps2.tile([128, CW], dt.float32, tag="s2")
                    for kp in range(KPAIR):
                        nc.tensor.matmul(
                            s2_ps[:],
                            zq_sb[:, 4 + 2 * kp + bt, :],
                            cmov[:, 2 * kp : 2 * kp + 2, w * CW : (w + 1) * CW],
                            start=(kp == 0),
                            stop=(kp == KPAIR - 1),
                            perf_mode=DRI,
                        )
                    nc.vector.tensor_copy(s2_sb[:, bt, ch, :], s2_ps[:])
                if ch == 7:
                    # first 8 chunk-columns done for both bt: export early so
                    # the write overlaps part-1 compute (sync ring, behind
                    # the qt input chunks)
                    for bt2 in range(2):
                        nc.sync.dma_start(
                            s2_d[bt2][:, 0:8, :].rearrange("p c w -> p (c w)"),
                            s2_sb[:, bt2, 0:8, :].rearrange("p c w -> p (c w)"),
                        )
            for bt in range(2):
                nc.sync.dma_start(
                    s2_d[bt][:, 8:CCH, :].rearrange("p c w -> p (c w)"),
                    s2_sb[:, bt, 8:CCH, :].rearrange("p c w -> p (c w)"),
                )

            # ---- part 1: queue shard, 16 big tiles of 512 rows ----
            # rowmax/s0 are taken from the fp8 exp tiles (exp is monotone;
            # the host works in log domain) so ACT's exp is the only PSUM
            # reader and the PE pipeline never waits on DVE.
            fin_sb = cpool.tile([128, 6, 64], dt.float32)   # rm | s0 | p1sum
            p1s_ps = ps1s.tile([128, B], dt.float32)        # sum-of-exp accumulator

            # ones-matmuls run one big-tile behind the s1 matmuls so the
            # in-order PE never waits for ACT's exp of the current tile
            exp_tiles = [None] * NBT
            for t in range(NBT):
                s1_ps = ps1.tile([128, 4, B], dt.float32, tag="s1")
                for q in range(4):
                    jt = t * 4 + q
                    h = next(
                        hh for hh in range(len(QSUBS) - 1, -1, -1) if jt >= QOFF[hh]
                    )
                    jl = jt - QOFF[h]
                    for kp in range(KPAIR):
                        nc.tensor.matmul(
                            s1_ps[:, q, :],
                            qt_sb[h][:, jl, kp, :],
                            zq_sb[:, 2 * kp : 2 * kp + 2, :],
                            start=(kp == 0),
                            stop=(kp == KPAIR - 1),
                            perf_mode=DRI,
                        )
                exp_t = wpool.tile([128, 4, B], dt.bfloat16, tag="exp")
                exp_tiles[t] = exp_t
                nc.scalar.activation(
                    exp_t[:],
                    s1_ps[:],
                    mybir.ActivationFunctionType.Exp,
                    scale=1.0 / (DOT_SCALE * INFO_TEMP),
                )
                if t > 1:
                    # two tiles behind: ACT's exp has a full tile of slack
                    for g in range(4):
                        nc.tensor.matmul(
                            p1s_ps[:],
                            ones_sb[:],
                            exp_tiles[t - 2][:, g, :],
                            start=(t == 2 and g == 0),
                            stop=False,
                        )
                nc.vector.tensor_reduce(
                    fin_sb[:, 0, t * 4 : t * 4 + 4],
                    exp_t[:],
                    axis=mybir.AxisListType.X,
                    op=mybir.AluOpType.max,
                )
                nc.vector.tensor_copy(fin_sb[:, 1, t * 4 : t * 4 + 4], exp_t[:, :, 0])
            for t in (NBT - 2, NBT - 1):
                for g in range(4):
                    nc.tensor.matmul(
                        p1s_ps[:],
                        ones_sb[:],
                        exp_tiles[t][:, g, :],
                        start=False,
                        stop=(t == NBT - 1 and g == 3),
                    )

            nc.vector.tensor_copy(
                fin_sb[:, 2:6].rearrange("p r q -> p (r q)"), p1s_ps[:]
            )
            nc.sync.dma_start(fin_d[:], fin_sb[:].rearrange("p r q -> p (r q)"))

    nc.compile()
    return nc


def _get_nc():
    if "nc" not in _CACHE:
        _CACHE["nc"] = _build()
    return _CACHE["nc"]


def _to_fp8(x):
    return (x * FP8_SCALE).astype(ml_dtypes.float8_e4m3fn)


def _interleave(A, B):
    """SwInterleave weight layout: mem[p, 2*jj+i] = layer_i[p, 127-jj].
    A, B: [..., 128, 128] (partition, column)."""
    return np.stack([A[..., ::-1], B[..., ::-1]], axis=-1).reshape(
        *A.shape[:-1], 256
    )


def _prep_inputs(Z_q, queue, centroids):
    """Host-side shard prep: x16 scale + e4m3 quantization + transpose to
    [C, rows], then partition-major chunk layouts so each DMA is a flat
    [128, N].  Stationary operands are pre-interleaved for SwInterleave."""
    zqT8 = _to_fp8(Z_q).T                            # [512, 256]
    zqT = zqT8.reshape(KSUB, 128, B).transpose(1, 0, 2)  # [128, KSUB, B]
    # part-2 stationary: [128, kp*2+bt, 256] interleaved
    zz = zqT8.reshape(KPAIR, 2, 128, 2, 128)         # [kp, i, p, bt, col]
    zqTi = (
        _interleave(zz[:, 0], zz[:, 1])
        .transpose(1, 0, 2, 3)
        .reshape(128, KSUB, 256)
    )
    zq = np.ascontiguousarray(np.concatenate([zqT, zqTi], axis=1))  # [128, 8, 256]

    qT = np.ascontiguousarray(_to_fp8(queue).T)      # [512, 65536]
    cT = np.ascontiguousarray(_to_fp8(centroids).T)  # [512, 50000]

    in_maps = []
    for i in range(NCORES):
        q_sh = qT[:, i * QSH : (i + 1) * QSH]        # [512, 8192]
        # [kp, i, p, jt, col]
        qq = q_sh.reshape(KPAIR, 2, 128, QSH // 128, 128)
        q_all = _interleave(qq[:, 0], qq[:, 1]).transpose(1, 2, 0, 3)
        # [128, 64, KPAIR, 256] -> variable-size chunks
        q_chunks = {
            f"qT{h}": np.ascontiguousarray(q_all[:, QOFF[h] : QOFF[h] + n])
            for h, n in enumerate(QSUBS)
        }
        c_sh = np.zeros((C, CSH_PAD), ml_dtypes.float8_e4m3fn)
        c_sh[:, :CSH] = cT[:, i * CSH : (i + 1) * CSH]
        # cTb = first 2 matmul chunks (small, lands first); cTa = the rest
        c_b = np.ascontiguousarray(
            c_sh[:, : 2 * CW].reshape(KSUB, 128, 2 * CW).transpose(1, 0, 2)
        )  # [128, KSUB, 2*CW]
        c_a = np.ascontiguousarray(
            c_sh[:, 2 * CW :].reshape(KSUB, 128, 3, 4 * CW).transpose(2, 1, 0, 3)
        )  # [3, 128, KSUB, 4*CW]
        in_maps.append({"zq": zq, "cTa": c_a, "cTb": c_b, **q_chunks})
    return in_maps


def kernel(Z_q, Z_k, queue, centroids, kmeans_temp, neg_raw):
    global last_exec_time_ns
    from concourse.bass_utils import run_bass_kernel_spmd

    nc = _get_nc()
    in_maps = _prep_inputs(Z_q, queue, centroids)

    trace = bool(int(os.environ.get("MOCO_BASS_TRACE", "0")))
    out = run_bass_kernel_spmd(nc, in_maps, core_ids=list(range(NCORES)), trace=trace)
    last_exec_time_ns = out.exec_time_ns
    res = out.results

    # ---- host combine (tiny) ----
    lp = (Z_q.astype(np.float64) * Z_k.astype(np.float64)).sum(axis=1)  # l_pos
    lp_t = lp / INFO_TEMP

    # part-1 loss: logsumexp over [l_pos | l_neg]/T per batch row.
    # Device partials are unshifted sums of e^{s/T} (|s/T| <= ~4).
    S = np.zeros(B, np.float64)
    for r in res:
        S += r["fin"][0, 128:].astype(np.float64)
    S += np.exp(lp_t)
    lse1 = np.log(S)
    loss1 = np.mean(lse1 - lp_t)

    # accuracy: exact despite fp8 scores.  Device exports per-queue-row
    # (max over batch, batch-0 score); every row with margin < MARGIN is
    # re-checked on the host in full precision.
    rm_full = np.empty(QUEUE, np.float64)
    s0_full = np.empty(QUEUE, np.float64)
    for i, r in enumerate(res):
        # device exports exp(s/T) values; recover s = T*log(.)
        # [128, 64] -> queue row j = (t*4+q)*128 + p within the shard
        fin = r["fin"].astype(np.float64)
        rm_full[i * QSH : (i + 1) * QSH] = np.log(fin[:, 0:64].T.reshape(-1)) * INFO_TEMP
        s0_full[i * QSH : (i + 1) * QSH] = np.log(fin[:, 64:128].T.reshape(-1)) * INFO_TEMP

    cand = (rm_full - s0_full) < MARGIN
    cols = np.nonzero(cand)[0]
    sub = Z_q.astype(np.float64) @ queue[cols].astype(np.float64).T  # [B, ncand]
    count = float((sub[0] >= sub.max(axis=0)).sum())
    count += float(lp[0] >= lp.max())
    accuracy = count / (1 + QUEUE)

    # part-2: global argmax over centroids (== argmin of ||c||^2 - 2 s).
    # s2 arrives in fp8; the argmax (and the positive logit) is resolved
    # exactly by re-checking every near-max column in full precision.
    s2_full = np.empty((B, NCL), np.float32)
    for i, r in enumerate(res):
        sh = r["s2"].astype(np.float32).reshape(B, CSH_PAD)    # [2,128,CCH,CW]
        s2_full[:, i * CSH : (i + 1) * CSH] = sh[:, :CSH]
    s2_full /= DOT_SCALE

    kt = kmeans_temp.astype(np.float64)
    Zq64 = Z_q.astype(np.float64)
    ce64 = centroids.astype(np.float64)
    mx = s2_full.max(axis=1)
    I = np.empty(B, np.int64)
    pl_pos = np.empty(B)
    for b in range(B):
        cnd = np.nonzero(s2_full[b] >= mx[b] - MARGIN2)[0]
        ex = ce64[cnd] @ Zq64[b]
        k = int(np.argmax(ex))
        I[b] = cnd[k]
        pl_pos[b] = ex[k] / kt[cnd[k]]

    neg_idx = neg_raw + (neg_raw >= I[:, None]).astype(neg_raw.dtype)
    pl_neg = (
        np.take_along_axis(s2_full, neg_idx, axis=1).astype(np.float64)
        / kt[neg_idx]
    )
    plogits = np.concatenate([pl_pos[:, None], pl_neg], axis=1)
    m = plogits.max(axis=1)
    plse = np.log(np.exp(plogits - m[:, None]).sum(axis=1)) + m
    ploss = np.mean(plse - pl_pos)

    loss = loss1 + PROTO_FACTOR * ploss
    return np.float32(loss), np.float32(accuracy)
